# revision 22
# baseline (speedup 1.0000x reference)
"""MultiHeadedAttention block (B=4, S=2048, D=1024, H=16) on 8 TRN2 cores.

Sharding: core c handles batch b=c//2 and query-row half c%2 (1024 rows).
Each core computes full K/V projections for its batch (2x redundant within a
batch pair), attention for all 16 heads over its 1024 query rows, then
O-projection + residual + LayerNorm. No collectives.

Device layouts (per core):
  Q^T  [o=1024, r=1024]  feature-major (partitions = features), per-ot tiles
  K^T  [o, k] projected per head pair inside the attention loop (no spill)
  V    [k=2048, o=1024]  row-major per-rt tiles, with a ones column per head
  scores computed transposed: S_t[k, q] = K_h^T Q_h  (softmax along k =
  partitions; exp without max-subtraction is safe: |logits| < ~3).
  P@V with the ones-augmented V gives the softmax denominator as row DK;
  normalization multiplies by a DMA-broadcast reciprocal. The V bias is
  exact through the normalization (bv*denom/denom), so it is added
  per-partition after normalizing.
All matmuls run in float32r (full PE rate at moving dim >= 256).
"""

import sys

if "/opt/trn_rl_repo" not in sys.path:
    sys.path.insert(0, "/opt/trn_rl_repo")

import ml_dtypes
import numpy as np

import concourse.bass as bass
import concourse.mybir as mybir
import concourse.tile as tile
from concourse.bass_utils import run_bass_kernel_spmd

B, S, D, H, DK = 4, 2048, 1024, 16, 64
P = 128
M = S // 2          # query rows per core
NDT = D // P        # 8 contraction chunks
NOT = D // P        # 8 output-feature chunks (= head pairs)
NHP = H // 2        # 8 head pairs
NKT = S // P        # 16 key chunks
NQT = M // 512      # 2 query 512-chunks
NRT_K = S // 512    # 4 key-row 512-chunks
NRT_V = S // P      # 16 V row chunks
NRT_O = M // P      # 8 output row chunks
KG = 2              # k-chunks per exp group
F32 = mybir.dt.float32
MM_DT = mybir.dt.float32r
AF = mybir.ActivationFunctionType
ALU = mybir.AluOpType


def _split_sync_waits(nc, max_waits=1):
    """Split instructions carrying more than max_waits sem waits.

    The container's walrus rejects instructions with multiple sync wait
    commands, so excess waits move onto NoOp instructions inserted just
    before, on the same engine.
    """
    idx = 0
    for f in nc.m.functions:
        for blk in f.blocks:
            newl = []
            for inst in blk.instructions:
                si = inst.sync_info
                waits = list(si.on_wait) if si is not None and si.on_wait else []
                if len(waits) > max_waits:
                    extra = waits[max_waits:]
                    si.on_wait = waits[:max_waits]
                    for j in range(0, len(extra), max_waits):
                        nop = mybir.InstNoOp(name=f"I-wsplit-{idx}", ins=[], outs=[])
                        idx += 1
                        nop.engine = inst.engine
                        nop.sync_info = mybir.SyncInfo(
                            on_wait=extra[j : j + max_waits], on_update=[]
                        )
                        newl.append(nop)
                newl.append(inst)
            blk.instructions = newl


def build_nc(loops=0):
    nc = bass.Bass()
    xqT = nc.dram_tensor("xqT", [D, M], F32, kind="ExternalInput")
    xkT = nc.dram_tensor("xkT", [D, S], F32, kind="ExternalInput")
    xvT = nc.dram_tensor("xvT", [D, S], F32, kind="ExternalInput")
    qres = nc.dram_tensor("qres", [M, D], F32, kind="ExternalInput")
    WqT = nc.dram_tensor("WqT", [D, D], F32, kind="ExternalInput")
    WkT = nc.dram_tensor("WkT", [D, D], F32, kind="ExternalInput")
    WvT = nc.dram_tensor("WvT", [D, D], F32, kind="ExternalInput")
    WoT = nc.dram_tensor("WoT", [D, D], F32, kind="ExternalInput")
    bqv = nc.dram_tensor("bq", [D], F32, kind="ExternalInput")
    bkv = nc.dram_tensor("bk", [D], F32, kind="ExternalInput")
    bvv = nc.dram_tensor("bv", [D], F32, kind="ExternalInput")
    gv = nc.dram_tensor("ln_g", [D], F32, kind="ExternalInput")
    bv2 = nc.dram_tensor("ln_b", [D], F32, kind="ExternalInput")
    onesv = nc.dram_tensor("onesv", [NRT_V * H], mybir.dt.bfloat16, kind="ExternalInput")
    onesf = nc.dram_tensor("onesf", [DK], F32, kind="ExternalInput")
    out = nc.dram_tensor("out", [M, D], F32, kind="ExternalOutput")

    WqT_r = WqT[:, :].rearrange("(a p) o -> p a o", p=P).bitcast(MM_DT)
    WkT_r = WkT[:, :].rearrange("(a p) o -> p a o", p=P).bitcast(MM_DT)
    WvT_r = WvT[:, :].rearrange("(a p) o -> p a o", p=P).bitcast(MM_DT)
    WoT_r = WoT[:, :].rearrange("(a p) o -> p a o", p=P).bitcast(MM_DT)
    xqT_r = xqT[:, :].rearrange("(a p) r -> p a r", p=P).bitcast(MM_DT)
    xkT_r = xkT[:, :].rearrange("(a p) r -> p a r", p=P).bitcast(MM_DT)
    xvT_r = xvT[:, :].rearrange("(a p) r -> p a r", p=P).bitcast(MM_DT)

    import contextlib

    with tile.TileContext(nc) as tc:
        loop_cm = tc.For_i(0, loops, 1) if loops else contextlib.nullcontext()
        loop_cm.__enter__()
        pxo_cm = tc.tile_pool(name="pxo", bufs=1)
        pxo = pxo_cm.__enter__()
        with (
            tc.tile_pool(name="pqv", bufs=1) as pqv,
        ):
            XO = [
                pxo.tile([P, M], MM_DT, tag=f"XO{i}", name=f"XO{i}")
                for i in range(NHP)
            ]

            QT = []
            for ot in range(NOT):
                t = pqv.tile([P, M], MM_DT, tag=f"QT{ot}", name=f"QT{ot}")
                QT.append(t)
            Vt = []
            for rt in range(NRT_V):
                t = pqv.tile([P, H, DK + 1], mybir.dt.bfloat16, tag=f"Vt{rt}", name=f"Vt{rt}")
                nc.sync.dma_start(
                    t[:, :, DK : DK + 1],
                    onesv[rt * H : (rt + 1) * H].partition_broadcast(P),
                )
                Vt.append(t)
            ones_t = pqv.tile([1, DK], MM_DT)
            nc.sync.dma_start(
                ones_t, onesf[:].partition_broadcast(1).bitcast(MM_DT)
            )
            bq_p = pqv.tile([P, NOT], F32)
            bk_p = pqv.tile([P, NOT], F32)
            bv_p = pqv.tile([P, NOT], F32)
            nc.sync.dma_start(bq_p, bqv[:].rearrange("(a p) -> p a", p=P))
            nc.sync.dma_start(bk_p, bkv[:].rearrange("(a p) -> p a", p=P))
            nc.sync.dma_start(bv_p, bvv[:].rearrange("(a p) -> p a", p=P))

            # wv loads early so phase B starts without a DMA stall
            pwv_cm = tc.tile_pool(name="pwv", bufs=NDT, side="right")
            pwv = pwv_cm.__enter__()
            wv = []
            for dt in range(NDT):
                w_t = pwv.tile([P, D], MM_DT, tag="wv", name=f"wv{dt}")
                nc.gpsimd.dma_start(w_t, WvT_r[:, dt, :])
                wv.append(w_t)

            pbx_cm = tc.tile_pool(name="pbx", bufs=3, side="right")
            pbx = pbx_cm.__enter__()
            psAB_cm = tc.tile_pool(name="psAB", bufs=6, space="PSUM")
            psAB = psAB_cm.__enter__()

            # ---- Phase A: Q^T = (Wq/8) @ x_q^T + bq/8, layout [o, r]
            with (
                tc.tile_pool(name="pa", bufs=NDT) as pa,
            ):
                wq = []
                xq = []
                for dt in range(NDT):
                    w_t = pa.tile([P, D], MM_DT, tag="wq", name=f"wq{dt}")
                    nc.sync.dma_start(w_t, WqT_r[:, dt, :])
                    wq.append(w_t)
                    x_t = pa.tile([P, M], MM_DT, tag="xq", name=f"xq{dt}")
                    nc.sync.dma_start(x_t, xqT_r[:, dt, :])
                    xq.append(x_t)
                for ot in range(NOT):
                    for qt in range(NQT):
                        ps = psAB.tile([P, 512], F32, tag='ps', name='ps')
                        for dt in range(NDT):
                            nc.tensor.matmul(
                                ps,
                                wq[dt][:, ot * P : (ot + 1) * P],
                                xq[dt][:, qt * 512 : (qt + 1) * 512],
                                start=(dt == 0),
                                stop=(dt == NDT - 1),
                            )
                        nc.vector.tensor_scalar_add(
                            QT[ot][:, qt * 512 : (qt + 1) * 512],
                            ps,
                            bq_p[:, ot : ot + 1],
                        )

            # xk loads during phase B so phase D starts without a DMA stall
            pdx_cm = tc.tile_pool(name="pdx", bufs=NDT)
            pdx = pdx_cm.__enter__()
            xk = []
            for dt in range(NDT):
                x_t = pdx.tile([P, S], MM_DT, tag="xk", name=f"xk{dt}")
                nc.gpsimd.dma_start(x_t, xkT_r[:, dt, :])
                xk.append(x_t)

            # ---- Phase B: V = x_v @ Wv^T (bias folded in later), [r, o]
            if True:
                for rt in range(NRT_V):
                    xv = pbx.tile([P, NDT, P], MM_DT)
                    nc.gpsimd.dma_start(xv, xvT_r[:, :, rt * P : (rt + 1) * P])
                    for o2 in range(2):
                        ps = psAB.tile([P, 512], F32, tag='ps', name='ps')
                        for dt in range(NDT):
                            nc.tensor.matmul(
                                ps,
                                xv[:, dt, :],
                                wv[dt][:, o2 * 512 : (o2 + 1) * 512],
                                start=(dt == 0),
                                stop=(dt == NDT - 1),
                            )
                        nc.vector.tensor_copy(
                            Vt[rt][:, o2 * 8 : (o2 + 1) * 8, 0:DK],
                            ps[:, :].rearrange("p (h e) -> p h e", e=DK),
                        )

            pbx_cm.__exit__(None, None, None)
            pwv_cm.__exit__(None, None, None)
            psAB_cm.__exit__(None, None, None)

            # ---- Phase D: K^T projection fused with attention, per head pair
            with (
                tc.tile_pool(name="pdw", bufs=2) as pdw,
                tc.tile_pool(name="pdkt", bufs=2) as pdkt,
                tc.tile_pool(name="pde", bufs=2) as pde,
                tc.tile_pool(name="pdr", bufs=1) as pdr,
                tc.tile_pool(name="psS", bufs=1, space="PSUM") as psS,
                tc.tile_pool(name="psK", bufs=1, space="PSUM") as psK,
                tc.tile_pool(name="psR", bufs=1, space="PSUM") as psR,
                tc.tile_pool(name="psPV", bufs=1, space="PSUM") as psPV,
            ):
                kts = {}

                def kproj(hp):
                    wk = pdw.tile([P, NDT, P], MM_DT, tag="wk", name="wk")
                    nc.sync.dma_start(wk, WkT_r[:, :, hp * P : (hp + 1) * P])
                    kt_t = pdkt.tile([P, S], MM_DT, tag="kt", name="kt")
                    for rt in range(NRT_K):
                        ps = psK.tile([P, 512], F32, tag="kps", name="kps")
                        for dt in range(NDT):
                            nc.tensor.matmul(
                                ps,
                                wk[:, dt, :],
                                xk[dt][:, rt * 512 : (rt + 1) * 512],
                                start=(dt == 0),
                                stop=(dt == NDT - 1),
                            )
                        nc.vector.tensor_scalar_add(
                            kt_t[:, rt * 512 : (rt + 1) * 512],
                            ps,
                            bk_p[:, hp : hp + 1],
                        )
                    kts[hp] = kt_t

                def attn(hp):
                    kt_t = kts.pop(hp)
                    xo_t = XO[hp]
                    for qt in range(NQT):
                        pv = [
                            psPV.tile(
                                [DK + 1, 512], F32, tag=f"pv{h01}", name=f"pv{h01}"
                            )
                            for h01 in range(2)
                        ]
                        for ktg in range(NKT // KG):
                            sss = [
                                psS.tile(
                                    [P, KG, 512], F32, tag=f"ss{h01}", name=f"ss{h01}"
                                )
                                for h01 in range(2)
                            ]
                            for j in range(KG):
                                kt = ktg * KG + j
                                for h01 in range(2):
                                    pb_ = h01 * DK
                                    nc.tensor.matmul(
                                        sss[h01][:, j, :],
                                        kt_t[pb_ : pb_ + DK, kt * P : (kt + 1) * P],
                                        QT[hp][
                                            pb_ : pb_ + DK,
                                            qt * 512 : (qt + 1) * 512,
                                        ],
                                        start=True,
                                        stop=True,
                                        tile_position=(pb_, 0),
                                    )
                            exs = []
                            for h01 in range(2):
                                ex = pde.tile(
                                    [P, KG, 512],
                                    mybir.dt.bfloat16,
                                    tag=f"ex{h01}",
                                    name=f"ex{h01}",
                                )
                                nc.scalar.activation(ex, sss[h01], AF.Exp)
                                exs.append(ex)
                            for h01 in range(2):
                                for j in range(KG):
                                    kt = ktg * KG + j
                                    nc.tensor.matmul(
                                        pv[h01],
                                        Vt[kt][:, 2 * hp + h01, :],
                                        exs[h01][:, j, :],
                                        start=(kt == 0),
                                        stop=(kt == NKT - 1),
                                    )
                        for h01 in range(2):
                            pb_ = h01 * DK
                            rc = pdr.tile([1, 512], MM_DT, tag="rc", name="rc")
                            with nc.allow_low_precision(
                                reason="1/denom feeds f32r broadcast matmul"
                            ):
                                nc.vector.reciprocal(rc, pv[h01][DK : DK + 1, :])
                            rbp = psR.tile([DK, 512], F32, tag="rbp", name="rbp")
                            nc.tensor.matmul(rbp, ones_t, rc, start=True, stop=True)
                            dst = xo_t[pb_ : pb_ + DK, qt * 512 : (qt + 1) * 512]
                            nc.vector.tensor_copy(dst, pv[h01][0:DK, :])
                            nc.vector.tensor_mul(dst, dst, rbp)
                            nc.vector.tensor_scalar_add(
                                dst, dst, bv_p[pb_ : pb_ + DK, hp : hp + 1]
                            )

                kproj(0)
                for hp in range(NHP):
                    if hp + 1 < NHP:
                        kproj(hp + 1)
                    attn(hp)

            pdx_cm.__exit__(None, None, None)

        # ---- Phase E: out = LN(x_o @ Wo^T + bo + q)  (bo pre-added to qres)
        with (
            tc.tile_pool(name="pe1", bufs=NDT) as pe1,
            tc.tile_pool(name="pec", bufs=1) as pec,
            tc.tile_pool(name="peq", bufs=2) as peq,
            tc.tile_pool(name="pey", bufs=3) as pey,
            tc.tile_pool(name="pst", bufs=8) as pst,
            tc.tile_pool(name="psE", bufs=4, space="PSUM") as psE,
        ):
            g_b = pec.tile([P, D], F32)
            b_b = pec.tile([P, D], F32)
            eps_t = pec.tile([P, 1], F32)
            nc.sync.dma_start(g_b, gv[:].partition_broadcast(P))
            nc.sync.dma_start(b_b, bv2[:].partition_broadcast(P))
            nc.vector.memset(eps_t, 1e-5)
            xo = XO
            wo = []
            for dt in range(NDT):
                w_t = pe1.tile([P, D], MM_DT, tag="wo", name=f"wo{dt}")
                nc.gpsimd.dma_start(w_t, WoT_r[:, dt, :])
                wo.append(w_t)
            for rt in range(NRT_O):
                qr = peq.tile([P, D], F32)
                nc.sync.dma_start(qr, qres[rt * P : (rt + 1) * P, :])
                y = pey.tile([P, D], F32)
                for o2 in range(2):
                    ps = psE.tile([P, 512], F32)
                    for hp in range(NOT):
                        nc.tensor.matmul(
                            ps,
                            xo[hp][:, rt * P : (rt + 1) * P],
                            wo[hp][:, o2 * 512 : (o2 + 1) * 512],
                            start=(hp == 0),
                            stop=(hp == NOT - 1),
                        )
                    nc.vector.tensor_add(
                        y[:, o2 * 512 : (o2 + 1) * 512],
                        ps,
                        qr[:, o2 * 512 : (o2 + 1) * 512],
                    )
                stats = pst.tile([P, 2, 6], F32)
                for sg in range(2):
                    nc.vector.bn_stats(
                        stats[:, sg, :], y[:, sg * 512 : (sg + 1) * 512]
                    )
                mv = pst.tile([P, 2], F32)
                nc.vector.bn_aggr(mv, stats)
                std = pst.tile([P, 1], F32)
                nc.scalar.activation(std, mv[:, 1:2], AF.Sqrt, bias=eps_t)
                rstd = pst.tile([P, 1], F32)
                nc.vector.reciprocal(rstd, std)
                nc.vector.tensor_scalar(
                    y,
                    y,
                    mv[:, 0:1],
                    rstd,
                    op0=ALU.subtract,
                    op1=ALU.mult,
                )
                nc.gpsimd.tensor_mul(y, y, g_b)
                nc.gpsimd.tensor_add(y, y, b_b)
                nc.sync.dma_start(out[rt * P : (rt + 1) * P, :], y)
        pxo_cm.__exit__(None, None, None)
        loop_cm.__exit__(None, None, None)
    _split_sync_waits(nc)
    return nc


_NC = None


def _get_nc():
    global _NC
    if _NC is None:
        _NC = build_nc()
    return _NC


def prepare_in_maps(q, k, v, Wq, bq, Wk, bk, Wv, bv, Wo, bo, ln_g, ln_b):
    f = np.float32
    q = np.asarray(q, f)
    k = np.asarray(k, f)
    v = np.asarray(v, f)
    scale = 1.0 / np.sqrt(np.float32(DK))
    WqT = np.ascontiguousarray(np.asarray(Wq, f).T * scale)
    WkT = np.ascontiguousarray(np.asarray(Wk, f).T)
    WvT = np.ascontiguousarray(np.asarray(Wv, f).T)
    WoT = np.ascontiguousarray(np.asarray(Wo, f).T)
    bq_s = np.asarray(bq, f) * scale
    common = {
        "WqT": WqT,
        "WkT": WkT,
        "WvT": WvT,
        "WoT": WoT,
        "bq": bq_s,
        "bk": np.asarray(bk, f),
        "bv": np.asarray(bv, f),
        "ln_g": np.asarray(ln_g, f),
        "ln_b": np.asarray(ln_b, f),
        "onesv": np.ones(NRT_V * H, ml_dtypes.bfloat16),
        "onesf": np.ones(DK, np.float32),
    }
    in_maps = []
    for c in range(8):
        b_, half = divmod(c, 2)
        qs = q[b_, half * M : (half + 1) * M, :]
        qres_c = qs + np.asarray(bo, f)[None, :]
        in_maps.append(
            dict(
                common,
                xqT=np.ascontiguousarray(qs.T),
                xkT=np.ascontiguousarray(k[b_].T),
                xvT=np.ascontiguousarray(v[b_].T),
                qres=np.ascontiguousarray(qres_c),
            )
        )
    return in_maps


def kernel(q, k, v, Wq, bq, Wk, bk, Wv, bv, Wo, bo, ln_g, ln_b):
    nc = _get_nc()
    in_maps = prepare_in_maps(q, k, v, Wq, bq, Wk, bk, Wv, bv, Wo, bo, ln_g, ln_b)
    res = run_bass_kernel_spmd(nc, in_maps, core_ids=list(range(8)))
    out = np.empty((B, S, D), np.float32)
    for c in range(8):
        b_, half = divmod(c, 2)
        out[b_, half * M : (half + 1) * M, :] = res.results[c]["out"]
    return out


# revision 23
# speedup vs baseline: 24.0529x; 24.0529x over previous
"""MultiHeadedAttention block (B=4, S=2048, D=1024, H=16) on 8 TRN2 cores.

Sharding: core c handles batch b=c//2 and query-row half c%2 (1024 rows).
Each core computes full K/V projections for its batch (2x redundant within a
batch pair), attention for all 16 heads over its 1024 query rows, then
O-projection + residual + LayerNorm. No collectives.

Device layouts (per core):
  Q^T  [o=1024, r=1024]  feature-major (partitions = features), per-ot tiles
  K^T  [o, k] projected per head pair inside the attention loop (no spill)
  V    [k=2048, o=1024]  row-major per-rt tiles, with a ones column per head
  scores computed transposed: S_t[k, q] = K_h^T Q_h  (softmax along k =
  partitions; exp without max-subtraction is safe: |logits| < ~3).
  P@V with the ones-augmented V gives the softmax denominator as row DK;
  normalization multiplies by a DMA-broadcast reciprocal. The V bias is
  exact through the normalization (bv*denom/denom), so it is added
  per-partition after normalizing.
All matmuls run in float32r (full PE rate at moving dim >= 256).
"""

import sys

if "/opt/trn_rl_repo" not in sys.path:
    sys.path.insert(0, "/opt/trn_rl_repo")

import ml_dtypes
import numpy as np

import concourse.bass as bass
import concourse.mybir as mybir
import concourse.tile as tile
from concourse.bass_utils import run_bass_kernel_spmd

B, S, D, H, DK = 4, 2048, 1024, 16, 64
P = 128
M = S // 2          # query rows per core
NDT = D // P        # 8 contraction chunks
NOT = D // P        # 8 output-feature chunks (= head pairs)
NHP = H // 2        # 8 head pairs
NKT = S // P        # 16 key chunks
NQT = M // 512      # 2 query 512-chunks
NRT_K = S // 512    # 4 key-row 512-chunks
NRT_V = S // P      # 16 V row chunks
NRT_O = M // P      # 8 output row chunks
KG = 2              # k-chunks per exp group
F32 = mybir.dt.float32
MM_DT = mybir.dt.float32r
AF = mybir.ActivationFunctionType
ALU = mybir.AluOpType


def _split_sync_waits(nc, max_waits=1):
    """Split instructions carrying more than max_waits sem waits.

    The container's walrus rejects instructions with multiple sync wait
    commands, so excess waits move onto NoOp instructions inserted just
    before, on the same engine.
    """
    idx = 0
    for f in nc.m.functions:
        for blk in f.blocks:
            newl = []
            for inst in blk.instructions:
                si = inst.sync_info
                waits = list(si.on_wait) if si is not None and si.on_wait else []
                if len(waits) > max_waits:
                    extra = waits[max_waits:]
                    si.on_wait = waits[:max_waits]
                    for j in range(0, len(extra), max_waits):
                        nop = mybir.InstNoOp(name=f"I-wsplit-{idx}", ins=[], outs=[])
                        idx += 1
                        nop.engine = inst.engine
                        nop.sync_info = mybir.SyncInfo(
                            on_wait=extra[j : j + max_waits], on_update=[]
                        )
                        newl.append(nop)
                newl.append(inst)
            blk.instructions = newl


def build_nc(loops=0):
    nc = bass.Bass()
    xqT = nc.dram_tensor("xqT", [D, M], F32, kind="ExternalInput")
    xkT = nc.dram_tensor("xkT", [D, S], F32, kind="ExternalInput")
    xvT = nc.dram_tensor("xvT", [D, S], F32, kind="ExternalInput")
    qres = nc.dram_tensor("qres", [M, D], F32, kind="ExternalInput")
    WqT = nc.dram_tensor("WqT", [D, D], F32, kind="ExternalInput")
    WkT = nc.dram_tensor("WkT", [D, D], F32, kind="ExternalInput")
    WvT = nc.dram_tensor("WvT", [D, D], F32, kind="ExternalInput")
    WoT = nc.dram_tensor("WoT", [D, D], F32, kind="ExternalInput")
    bqv = nc.dram_tensor("bq", [D], F32, kind="ExternalInput")
    bkv = nc.dram_tensor("bk", [D], F32, kind="ExternalInput")
    bvv = nc.dram_tensor("bv", [D], F32, kind="ExternalInput")
    gv = nc.dram_tensor("ln_g", [D], F32, kind="ExternalInput")
    bv2 = nc.dram_tensor("ln_b", [D], F32, kind="ExternalInput")
    onesv = nc.dram_tensor("onesv", [NRT_V * H], mybir.dt.bfloat16, kind="ExternalInput")
    onesf = nc.dram_tensor("onesf", [DK], F32, kind="ExternalInput")
    out = nc.dram_tensor("out", [M, D], F32, kind="ExternalOutput")

    WqT_r = WqT[:, :].rearrange("(a p) o -> p a o", p=P).bitcast(MM_DT)
    WkT_r = WkT[:, :].rearrange("(a p) o -> p a o", p=P).bitcast(MM_DT)
    WvT_r = WvT[:, :].rearrange("(a p) o -> p a o", p=P).bitcast(MM_DT)
    WoT_r = WoT[:, :].rearrange("(a p) o -> p a o", p=P).bitcast(MM_DT)
    xqT_r = xqT[:, :].rearrange("(a p) r -> p a r", p=P).bitcast(MM_DT)
    xkT_r = xkT[:, :].rearrange("(a p) r -> p a r", p=P).bitcast(MM_DT)
    xvT_r = xvT[:, :].rearrange("(a p) r -> p a r", p=P).bitcast(MM_DT)

    import contextlib

    with tile.TileContext(nc) as tc:
        loop_cm = tc.For_i(0, loops, 1) if loops else contextlib.nullcontext()
        loop_cm.__enter__()
        pxo_cm = tc.tile_pool(name="pxo", bufs=1)
        pxo = pxo_cm.__enter__()
        with (
            tc.tile_pool(name="pqv", bufs=1) as pqv,
        ):
            XO = [
                pxo.tile([P, M], MM_DT, tag=f"XO{i}", name=f"XO{i}")
                for i in range(NHP)
            ]

            QT = []
            for ot in range(NOT):
                t = pqv.tile([P, M], MM_DT, tag=f"QT{ot}", name=f"QT{ot}")
                QT.append(t)
            Vt = []
            for rt in range(NRT_V):
                t = pqv.tile([P, H, DK + 1], mybir.dt.bfloat16, tag=f"Vt{rt}", name=f"Vt{rt}")
                nc.gpsimd.dma_start(
                    t[:, :, DK : DK + 1],
                    onesv[rt * H : (rt + 1) * H].partition_broadcast(P),
                )
                Vt.append(t)
            ones_t = pqv.tile([1, DK], MM_DT)
            nc.sync.dma_start(
                ones_t, onesf[:].partition_broadcast(1).bitcast(MM_DT)
            )
            bq_p = pqv.tile([P, NOT], F32)
            bk_p = pqv.tile([P, NOT], F32)
            bv_p = pqv.tile([P, NOT], F32)
            nc.gpsimd.dma_start(bq_p, bqv[:].rearrange("(a p) -> p a", p=P))
            nc.gpsimd.dma_start(bk_p, bkv[:].rearrange("(a p) -> p a", p=P))
            nc.gpsimd.dma_start(bv_p, bvv[:].rearrange("(a p) -> p a", p=P))

            # wv loads early so phase B starts without a DMA stall
            pwv_cm = tc.tile_pool(name="pwv", bufs=NDT, side="right")
            pwv = pwv_cm.__enter__()
            wv = []
            for dt in range(NDT):
                w_t = pwv.tile([P, D], MM_DT, tag="wv", name=f"wv{dt}")
                nc.gpsimd.dma_start(w_t, WvT_r[:, dt, :])
                wv.append(w_t)

            pbx_cm = tc.tile_pool(name="pbx", bufs=3, side="right")
            pbx = pbx_cm.__enter__()
            psAB_cm = tc.tile_pool(name="psAB", bufs=6, space="PSUM")
            psAB = psAB_cm.__enter__()

            # ---- Phase A: Q^T = (Wq/8) @ x_q^T + bq/8, layout [o, r]
            with (
                tc.tile_pool(name="pa", bufs=NDT) as pa,
            ):
                wq = []
                xq = []
                for dt in range(NDT):
                    w_t = pa.tile([P, D], MM_DT, tag="wq", name=f"wq{dt}")
                    nc.sync.dma_start(w_t, WqT_r[:, dt, :])
                    wq.append(w_t)
                    x_t = pa.tile([P, M], MM_DT, tag="xq", name=f"xq{dt}")
                    nc.sync.dma_start(x_t, xqT_r[:, dt, :])
                    xq.append(x_t)
                for ot in range(NOT):
                    for qt in range(NQT):
                        ps = psAB.tile([P, 512], F32, tag='ps', name='ps')
                        for dt in range(NDT):
                            nc.tensor.matmul(
                                ps,
                                wq[dt][:, ot * P : (ot + 1) * P],
                                xq[dt][:, qt * 512 : (qt + 1) * 512],
                                start=(dt == 0),
                                stop=(dt == NDT - 1),
                            )
                        nc.vector.tensor_scalar_add(
                            QT[ot][:, qt * 512 : (qt + 1) * 512],
                            ps,
                            bq_p[:, ot : ot + 1],
                        )

            # xk loads during phase B so phase D starts without a DMA stall
            pdx_cm = tc.tile_pool(name="pdx", bufs=NDT)
            pdx = pdx_cm.__enter__()
            xk = []
            for dt in range(NDT):
                x_t = pdx.tile([P, S], MM_DT, tag="xk", name=f"xk{dt}")
                nc.gpsimd.dma_start(x_t, xkT_r[:, dt, :])
                xk.append(x_t)

            # ---- Phase B: V = x_v @ Wv^T (bias folded in later), [r, o]
            if True:
                for rt in range(NRT_V):
                    xv = pbx.tile([P, NDT, P], MM_DT)
                    nc.gpsimd.dma_start(xv, xvT_r[:, :, rt * P : (rt + 1) * P])
                    for o2 in range(2):
                        ps = psAB.tile([P, 512], F32, tag='ps', name='ps')
                        for dt in range(NDT):
                            nc.tensor.matmul(
                                ps,
                                xv[:, dt, :],
                                wv[dt][:, o2 * 512 : (o2 + 1) * 512],
                                start=(dt == 0),
                                stop=(dt == NDT - 1),
                            )
                        nc.vector.tensor_copy(
                            Vt[rt][:, o2 * 8 : (o2 + 1) * 8, 0:DK],
                            ps[:, :].rearrange("p (h e) -> p h e", e=DK),
                        )

            pbx_cm.__exit__(None, None, None)
            pwv_cm.__exit__(None, None, None)
            psAB_cm.__exit__(None, None, None)

            # ---- Phase D: K^T projection fused with attention, per head pair
            with (
                tc.tile_pool(name="pdw", bufs=2) as pdw,
                tc.tile_pool(name="pdkt", bufs=2) as pdkt,
                tc.tile_pool(name="pde", bufs=2) as pde,
                tc.tile_pool(name="pdr", bufs=1) as pdr,
                tc.tile_pool(name="psS", bufs=1, space="PSUM") as psS,
                tc.tile_pool(name="psK", bufs=1, space="PSUM") as psK,
                tc.tile_pool(name="psR", bufs=1, space="PSUM") as psR,
                tc.tile_pool(name="psPV", bufs=1, space="PSUM") as psPV,
            ):
                kts = {}

                def kproj(hp):
                    wk = pdw.tile([P, NDT, P], MM_DT, tag="wk", name="wk")
                    nc.sync.dma_start(wk, WkT_r[:, :, hp * P : (hp + 1) * P])
                    kt_t = pdkt.tile([P, S], MM_DT, tag="kt", name="kt")
                    for rt in range(NRT_K):
                        ps = psK.tile([P, 512], F32, tag="kps", name="kps")
                        for dt in range(NDT):
                            nc.tensor.matmul(
                                ps,
                                wk[:, dt, :],
                                xk[dt][:, rt * 512 : (rt + 1) * 512],
                                start=(dt == 0),
                                stop=(dt == NDT - 1),
                            )
                        nc.vector.tensor_scalar_add(
                            kt_t[:, rt * 512 : (rt + 1) * 512],
                            ps,
                            bk_p[:, hp : hp + 1],
                        )
                    kts[hp] = kt_t

                def attn(hp):
                    kt_t = kts.pop(hp)
                    xo_t = XO[hp]
                    for qt in range(NQT):
                        pv = [
                            psPV.tile(
                                [DK + 1, 512], F32, tag=f"pv{h01}", name=f"pv{h01}"
                            )
                            for h01 in range(2)
                        ]
                        for ktg in range(NKT // KG):
                            sss = [
                                psS.tile(
                                    [P, KG, 512], F32, tag=f"ss{h01}", name=f"ss{h01}"
                                )
                                for h01 in range(2)
                            ]
                            for j in range(KG):
                                kt = ktg * KG + j
                                for h01 in range(2):
                                    pb_ = h01 * DK
                                    nc.tensor.matmul(
                                        sss[h01][:, j, :],
                                        kt_t[pb_ : pb_ + DK, kt * P : (kt + 1) * P],
                                        QT[hp][
                                            pb_ : pb_ + DK,
                                            qt * 512 : (qt + 1) * 512,
                                        ],
                                        start=True,
                                        stop=True,
                                        tile_position=(pb_, 0),
                                    )
                            exs = []
                            for h01 in range(2):
                                ex = pde.tile(
                                    [P, KG, 512],
                                    mybir.dt.bfloat16,
                                    tag=f"ex{h01}",
                                    name=f"ex{h01}",
                                )
                                nc.scalar.activation(ex, sss[h01], AF.Exp)
                                exs.append(ex)
                            for h01 in range(2):
                                for j in range(KG):
                                    kt = ktg * KG + j
                                    nc.tensor.matmul(
                                        pv[h01],
                                        Vt[kt][:, 2 * hp + h01, :],
                                        exs[h01][:, j, :],
                                        start=(kt == 0),
                                        stop=(kt == NKT - 1),
                                    )
                        for h01 in range(2):
                            pb_ = h01 * DK
                            rc = pdr.tile([1, 512], MM_DT, tag="rc", name="rc")
                            with nc.allow_low_precision(
                                reason="1/denom feeds f32r broadcast matmul"
                            ):
                                nc.vector.reciprocal(rc, pv[h01][DK : DK + 1, :])
                            rbp = psR.tile([DK, 512], F32, tag="rbp", name="rbp")
                            nc.tensor.matmul(rbp, ones_t, rc, start=True, stop=True)
                            dst = xo_t[pb_ : pb_ + DK, qt * 512 : (qt + 1) * 512]
                            nc.vector.tensor_copy(dst, pv[h01][0:DK, :])
                            nc.vector.tensor_mul(dst, dst, rbp)
                            nc.vector.tensor_scalar_add(
                                dst, dst, bv_p[pb_ : pb_ + DK, hp : hp + 1]
                            )

                kproj(0)
                for hp in range(NHP):
                    if hp + 1 < NHP:
                        kproj(hp + 1)
                    attn(hp)

            pdx_cm.__exit__(None, None, None)

        # ---- Phase E: out = LN(x_o @ Wo^T + bo + q)  (bo pre-added to qres)
        with (
            tc.tile_pool(name="pe1", bufs=NDT) as pe1,
            tc.tile_pool(name="pec", bufs=1) as pec,
            tc.tile_pool(name="peq", bufs=2) as peq,
            tc.tile_pool(name="pey", bufs=3) as pey,
            tc.tile_pool(name="pst", bufs=8) as pst,
            tc.tile_pool(name="psE", bufs=4, space="PSUM") as psE,
        ):
            g_b = pec.tile([P, D], F32)
            b_b = pec.tile([P, D], F32)
            eps_t = pec.tile([P, 1], F32)
            nc.sync.dma_start(g_b, gv[:].partition_broadcast(P))
            nc.sync.dma_start(b_b, bv2[:].partition_broadcast(P))
            nc.vector.memset(eps_t, 1e-5)
            xo = XO
            wo = []
            for dt in range(NDT):
                w_t = pe1.tile([P, D], MM_DT, tag="wo", name=f"wo{dt}")
                nc.gpsimd.dma_start(w_t, WoT_r[:, dt, :])
                wo.append(w_t)
            for rt in range(NRT_O):
                qr = peq.tile([P, D], F32)
                nc.sync.dma_start(qr, qres[rt * P : (rt + 1) * P, :])
                y = pey.tile([P, D], F32)
                for o2 in range(2):
                    ps = psE.tile([P, 512], F32)
                    for hp in range(NOT):
                        nc.tensor.matmul(
                            ps,
                            xo[hp][:, rt * P : (rt + 1) * P],
                            wo[hp][:, o2 * 512 : (o2 + 1) * 512],
                            start=(hp == 0),
                            stop=(hp == NOT - 1),
                        )
                    nc.vector.tensor_add(
                        y[:, o2 * 512 : (o2 + 1) * 512],
                        ps,
                        qr[:, o2 * 512 : (o2 + 1) * 512],
                    )
                stats = pst.tile([P, 2, 6], F32)
                for sg in range(2):
                    nc.vector.bn_stats(
                        stats[:, sg, :], y[:, sg * 512 : (sg + 1) * 512]
                    )
                mv = pst.tile([P, 2], F32)
                nc.vector.bn_aggr(mv, stats)
                std = pst.tile([P, 1], F32)
                nc.scalar.activation(std, mv[:, 1:2], AF.Sqrt, bias=eps_t)
                rstd = pst.tile([P, 1], F32)
                nc.vector.reciprocal(rstd, std)
                nc.vector.tensor_scalar(
                    y,
                    y,
                    mv[:, 0:1],
                    rstd,
                    op0=ALU.subtract,
                    op1=ALU.mult,
                )
                nc.gpsimd.tensor_mul(y, y, g_b)
                nc.gpsimd.tensor_add(y, y, b_b)
                nc.sync.dma_start(out[rt * P : (rt + 1) * P, :], y)
        pxo_cm.__exit__(None, None, None)
        loop_cm.__exit__(None, None, None)
    _split_sync_waits(nc)
    return nc


_NC = None


def _get_nc():
    global _NC
    if _NC is None:
        _NC = build_nc()
    return _NC


def prepare_in_maps(q, k, v, Wq, bq, Wk, bk, Wv, bv, Wo, bo, ln_g, ln_b):
    f = np.float32
    q = np.asarray(q, f)
    k = np.asarray(k, f)
    v = np.asarray(v, f)
    scale = 1.0 / np.sqrt(np.float32(DK))
    WqT = np.ascontiguousarray(np.asarray(Wq, f).T * scale)
    WkT = np.ascontiguousarray(np.asarray(Wk, f).T)
    WvT = np.ascontiguousarray(np.asarray(Wv, f).T)
    WoT = np.ascontiguousarray(np.asarray(Wo, f).T)
    bq_s = np.asarray(bq, f) * scale
    common = {
        "WqT": WqT,
        "WkT": WkT,
        "WvT": WvT,
        "WoT": WoT,
        "bq": bq_s,
        "bk": np.asarray(bk, f),
        "bv": np.asarray(bv, f),
        "ln_g": np.asarray(ln_g, f),
        "ln_b": np.asarray(ln_b, f),
        "onesv": np.ones(NRT_V * H, ml_dtypes.bfloat16),
        "onesf": np.ones(DK, np.float32),
    }
    in_maps = []
    for c in range(8):
        b_, half = divmod(c, 2)
        qs = q[b_, half * M : (half + 1) * M, :]
        qres_c = qs + np.asarray(bo, f)[None, :]
        in_maps.append(
            dict(
                common,
                xqT=np.ascontiguousarray(qs.T),
                xkT=np.ascontiguousarray(k[b_].T),
                xvT=np.ascontiguousarray(v[b_].T),
                qres=np.ascontiguousarray(qres_c),
            )
        )
    return in_maps


def kernel(q, k, v, Wq, bq, Wk, bk, Wv, bv, Wo, bo, ln_g, ln_b):
    nc = _get_nc()
    in_maps = prepare_in_maps(q, k, v, Wq, bq, Wk, bk, Wv, bv, Wo, bo, ln_g, ln_b)
    res = run_bass_kernel_spmd(nc, in_maps, core_ids=list(range(8)))
    out = np.empty((B, S, D), np.float32)
    for c in range(8):
        b_, half = divmod(c, 2)
        out[b_, half * M : (half + 1) * M, :] = res.results[c]["out"]
    return out


# revision 36
# speedup vs baseline: 27.4325x; 1.1405x over previous
"""MultiHeadedAttention block (B=4, S=2048, D=1024, H=16) on 8 TRN2 cores.

Sharding: core c handles batch b=c//2 and query-row half c%2 (1024 rows).
Each core computes full K/V projections for its batch (2x redundant within a
batch pair), attention for all 16 heads over its 1024 query rows, then
O-projection + residual + LayerNorm. No collectives.

Device layouts (per core):
  Q^T  [o=1024, r=1024]  feature-major (partitions = features), per-ot tiles
  K^T  [o, k] projected per head pair inside the attention loop (no spill)
  V    [k=2048, o=1024]  row-major per-rt tiles, with a ones column per head
  scores computed transposed: S_t[k, q] = K_h^T Q_h  (softmax along k =
  partitions; exp without max-subtraction is safe: |logits| < ~3).
  P@V with the ones-augmented V gives the softmax denominator as row DK;
  normalization multiplies by a DMA-broadcast reciprocal. The V bias is
  exact through the normalization (bv*denom/denom), so it is added
  per-partition after normalizing.
All matmuls run in float32r (full PE rate at moving dim >= 256).
"""

import sys

if "/opt/trn_rl_repo" not in sys.path:
    sys.path.insert(0, "/opt/trn_rl_repo")

import ml_dtypes
import numpy as np

import concourse.bass as bass
import concourse.mybir as mybir
import concourse.tile as tile
from concourse.bass_utils import run_bass_kernel_spmd

B, S, D, H, DK = 4, 2048, 1024, 16, 64
P = 128
M = S // 2          # query rows per core
NDT = D // P        # 8 contraction chunks
NOT = D // P        # 8 output-feature chunks (= head pairs)
NHP = H // 2        # 8 head pairs
NKT = S // P        # 16 key chunks
NQT = M // 512      # 2 query 512-chunks
NRT_K = S // 512    # 4 key-row 512-chunks
NRT_V = S // P      # 16 V row chunks
NRT_O = M // P      # 8 output row chunks
KG = 2              # k-chunks per exp group
F32 = mybir.dt.float32
MM_DT = mybir.dt.float32r
AF = mybir.ActivationFunctionType
ALU = mybir.AluOpType


def _split_sync_waits(nc, max_waits=1):
    """Split instructions carrying more than max_waits sem waits.

    The container's walrus rejects instructions with multiple sync wait
    commands, so excess waits move onto NoOp instructions inserted just
    before, on the same engine.
    """
    idx = 0
    for f in nc.m.functions:
        for blk in f.blocks:
            newl = []
            for inst in blk.instructions:
                si = inst.sync_info
                waits = list(si.on_wait) if si is not None and si.on_wait else []
                if len(waits) > max_waits:
                    extra = waits[max_waits:]
                    si.on_wait = waits[:max_waits]
                    for j in range(0, len(extra), max_waits):
                        nop = mybir.InstNoOp(name=f"I-wsplit-{idx}", ins=[], outs=[])
                        idx += 1
                        nop.engine = inst.engine
                        nop.sync_info = mybir.SyncInfo(
                            on_wait=extra[j : j + max_waits], on_update=[]
                        )
                        newl.append(nop)
                newl.append(inst)
            blk.instructions = newl


def build_nc(loops=0):
    nc = bass.Bass()
    xqT = nc.dram_tensor("xqT", [D, M], mybir.dt.bfloat16, kind="ExternalInput")
    xkT = nc.dram_tensor("xkT", [D, S], mybir.dt.bfloat16, kind="ExternalInput")
    xvT = nc.dram_tensor("xvT", [D, S], mybir.dt.bfloat16, kind="ExternalInput")
    qres = nc.dram_tensor("qres", [M, D], F32, kind="ExternalInput")
    WqT = nc.dram_tensor("WqT", [D, D], mybir.dt.bfloat16, kind="ExternalInput")
    WkT = nc.dram_tensor("WkT", [D, D], mybir.dt.bfloat16, kind="ExternalInput")
    WvT = nc.dram_tensor("WvT", [D, D], mybir.dt.bfloat16, kind="ExternalInput")
    WoT = nc.dram_tensor("WoT", [D, D], mybir.dt.bfloat16, kind="ExternalInput")
    bqv = nc.dram_tensor("bq", [D], F32, kind="ExternalInput")
    bkv = nc.dram_tensor("bk", [D], F32, kind="ExternalInput")
    bvv = nc.dram_tensor("bv", [D], F32, kind="ExternalInput")
    gv = nc.dram_tensor("ln_g", [D], F32, kind="ExternalInput")
    bv2 = nc.dram_tensor("ln_b", [D], F32, kind="ExternalInput")
    onesv = nc.dram_tensor("onesv", [P, NRT_V * H], mybir.dt.bfloat16, kind="ExternalInput")
    onesf = nc.dram_tensor("onesf", [DK], F32, kind="ExternalInput")
    out = nc.dram_tensor("out", [M, D], F32, kind="ExternalOutput")

    WqT_r = WqT[:, :].rearrange("(a p) o -> p a o", p=P)
    WkT_r = WkT[:, :].rearrange("(a p) o -> p a o", p=P)
    WvT_r = WvT[:, :].rearrange("(a p) o -> p a o", p=P)
    WoT_r = WoT[:, :].rearrange("(a p) o -> p a o", p=P)
    xqT_r = xqT[:, :].rearrange("(a p) r -> p a r", p=P)
    xkT_r = xkT[:, :].rearrange("(a p) r -> p a r", p=P)
    xvT_r = xvT[:, :].rearrange("(a p) r -> p a r", p=P)

    import contextlib

    with tile.TileContext(nc) as tc:
        loop_cm = tc.For_i(0, loops, 1) if loops else contextlib.nullcontext()
        loop_cm.__enter__()
        pxo_cm = tc.tile_pool(name="pxo", bufs=1)
        pxo = pxo_cm.__enter__()
        with (
            tc.tile_pool(name="pqv", bufs=1) as pqv,
        ):
            XO = [
                pxo.tile([P, M], mybir.dt.bfloat16, tag=f"XO{i}", name=f"XO{i}")
                for i in range(NHP)
            ]

            QT = []
            for ot in range(NOT):
                t = pqv.tile([P, M], mybir.dt.bfloat16, tag=f"QT{ot}", name=f"QT{ot}")
                QT.append(t)
            Vt = []
            for rt in range(NRT_V):
                t = pqv.tile([P, H, DK + 1], mybir.dt.bfloat16, tag=f"Vt{rt}", name=f"Vt{rt}")
                nc.gpsimd.dma_start(
                    t[:, :, DK : DK + 1],
                    onesv[:, rt * H : (rt + 1) * H],
                )
                Vt.append(t)
            ones_t = pqv.tile([1, DK], MM_DT)
            nc.gpsimd.dma_start(
                ones_t, onesf[:].partition_broadcast(1).bitcast(MM_DT)
            )
            bq_p = pqv.tile([P, NOT], F32)
            bk_p = pqv.tile([P, NOT], F32)
            bv_p = pqv.tile([P, NOT], F32)
            nc.gpsimd.dma_start(bq_p, bqv[:].rearrange("(a p) -> p a", p=P))
            nc.gpsimd.dma_start(bk_p, bkv[:].rearrange("(a p) -> p a", p=P))
            nc.gpsimd.dma_start(bv_p, bvv[:].rearrange("(a p) -> p a", p=P))

            # wv loads early so phase B starts without a DMA stall
            pwv_cm = tc.tile_pool(name="pwv", bufs=NDT, side="right")
            pwv = pwv_cm.__enter__()
            wv = []
            for dt in range(NDT):
                w_t = pwv.tile([P, D], mybir.dt.bfloat16, tag="wv", name=f"wv{dt}")
                nc.gpsimd.dma_start(w_t, WvT_r[:, dt, :])
                wv.append(w_t)

            pbx_cm = tc.tile_pool(name="pbx", bufs=3, side="right")
            pbx = pbx_cm.__enter__()
            psAB_cm = tc.tile_pool(name="psAB", bufs=8, space="PSUM")
            psAB = psAB_cm.__enter__()

            # ---- Phase A: Q^T = (Wq/8) @ x_q^T + bq/8, layout [o, r]
            with (
                tc.tile_pool(name="pa", bufs=NDT) as pa,
            ):
                wq = []
                xq = []
                xv_pre = {}
                for dt in range(NDT):
                    w_t = pa.tile([P, D], mybir.dt.bfloat16, tag="wq", name=f"wq{dt}")
                    nc.sync.dma_start(w_t, WqT_r[:, dt, :])
                    wq.append(w_t)
                    x_t = pa.tile([P, M], mybir.dt.bfloat16, tag="xq", name=f"xq{dt}")
                    nc.sync.dma_start(x_t, xqT_r[:, dt, :])
                    xq.append(x_t)
                    if dt in (2, 4, 6):
                        rt = dt // 2 - 1
                        xv_t = pbx.tile(
                            [P, NDT, P], mybir.dt.bfloat16, tag="xv", name="xv"
                        )
                        nc.sync.dma_start(
                            xv_t, xvT_r[:, :, rt * P : (rt + 1) * P]
                        )
                        xv_pre[rt] = xv_t
                for ot in range(NOT):
                    for qt in range(NQT):
                        ps = psAB.tile([P, 512], F32, tag='ps', name='ps')
                        for dt in range(NDT):
                            nc.tensor.matmul(
                                ps,
                                wq[dt][:, ot * P : (ot + 1) * P],
                                xq[dt][:, qt * 512 : (qt + 1) * 512],
                                start=(dt == 0),
                                stop=(dt == NDT - 1),
                            )
                        nc.vector.tensor_scalar_add(
                            QT[ot][:, qt * 512 : (qt + 1) * 512],
                            ps,
                            bq_p[:, ot : ot + 1],
                        )

            # xk loads during phase B so phase D starts without a DMA stall
            pdx_cm = tc.tile_pool(name="pdx", bufs=NDT)
            pdx = pdx_cm.__enter__()
            xk = []
            for dt in range(NDT):
                x_t = pdx.tile([P, S], mybir.dt.bfloat16, tag="xk", name=f"xk{dt}")
                nc.sync.dma_start(x_t, xkT_r[:, dt, :])
                xk.append(x_t)

            # ---- Phase B: V = x_v @ Wv^T (bias folded in later), [r, o]
            if True:
                for rt in range(NRT_V):
                    if rt in xv_pre:
                        xv = xv_pre.pop(rt)
                    else:
                        xv = pbx.tile(
                            [P, NDT, P], mybir.dt.bfloat16, tag="xv", name="xv"
                        )
                        nc.sync.dma_start(xv, xvT_r[:, :, rt * P : (rt + 1) * P])
                    for o2 in range(2):
                        ps = psAB.tile([P, 512], F32, tag='ps', name='ps')
                        for dt in range(NDT):
                            nc.tensor.matmul(
                                ps,
                                xv[:, dt, :],
                                wv[dt][:, o2 * 512 : (o2 + 1) * 512],
                                start=(dt == 0),
                                stop=(dt == NDT - 1),
                            )
                        nc.vector.tensor_copy(
                            Vt[rt][:, o2 * 8 : (o2 + 1) * 8, 0:DK],
                            ps[:, :].rearrange("p (h e) -> p h e", e=DK),
                        )

            pbx_cm.__exit__(None, None, None)
            pwv_cm.__exit__(None, None, None)
            psAB_cm.__exit__(None, None, None)

            # wo prefetch during D so phase E starts without a DMA stall
            pwo_cm = tc.tile_pool(name="pwo", bufs=NDT, side="right")
            pwo = pwo_cm.__enter__()
            wo = []
            for dt in range(NDT):
                w_t = pwo.tile([P, D], mybir.dt.bfloat16, tag="wo", name=f"wo{dt}")
                nc.gpsimd.dma_start(w_t, WoT_r[:, dt, :])
                wo.append(w_t)

            # ---- Phase D: K^T projection fused with attention, per head pair
            with (
                tc.tile_pool(name="pdw", bufs=2) as pdw,
                tc.tile_pool(name="pdkt", bufs=2) as pdkt,
                tc.tile_pool(name="pde", bufs=2) as pde,
                tc.tile_pool(name="pdr", bufs=1) as pdr,
                tc.tile_pool(name="psS", bufs=1, space="PSUM") as psS,
                tc.tile_pool(name="psK", bufs=1, space="PSUM") as psK,
                tc.tile_pool(name="psR", bufs=1, space="PSUM") as psR,
                tc.tile_pool(name="psPV", bufs=1, space="PSUM") as psPV,
            ):
                kts = {}

                def kproj(hp):
                    wk = pdw.tile([P, NDT, P], mybir.dt.bfloat16, tag="wk", name="wk")
                    nc.sync.dma_start(wk, WkT_r[:, :, hp * P : (hp + 1) * P])
                    kt_t = pdkt.tile([P, S], mybir.dt.bfloat16, tag="kt", name="kt")
                    for rt in range(NRT_K):
                        ps = psK.tile([P, 512], F32, tag="kps", name="kps")
                        for dt in range(NDT):
                            nc.tensor.matmul(
                                ps,
                                wk[:, dt, :],
                                xk[dt][:, rt * 512 : (rt + 1) * 512],
                                start=(dt == 0),
                                stop=(dt == NDT - 1),
                            )
                        nc.vector.tensor_scalar_add(
                            kt_t[:, rt * 512 : (rt + 1) * 512],
                            ps,
                            bk_p[:, hp : hp + 1],
                        )
                    kts[hp] = kt_t

                def attn(hp):
                    kt_t = kts.pop(hp)
                    xo_t = XO[hp]
                    for qt in range(NQT):
                        pv = [
                            psPV.tile(
                                [DK + 1, 512], F32, tag=f"pv{h01}", name=f"pv{h01}"
                            )
                            for h01 in range(2)
                        ]
                        for ktg in range(NKT // KG):
                            sss = [
                                psS.tile(
                                    [P, KG, 512], F32, tag=f"ss{h01}", name=f"ss{h01}"
                                )
                                for h01 in range(2)
                            ]
                            for j in range(KG):
                                kt = ktg * KG + j
                                for h01 in range(2):
                                    pb_ = h01 * DK
                                    nc.tensor.matmul(
                                        sss[h01][:, j, :],
                                        kt_t[pb_ : pb_ + DK, kt * P : (kt + 1) * P],
                                        QT[hp][
                                            pb_ : pb_ + DK,
                                            qt * 512 : (qt + 1) * 512,
                                        ],
                                        start=True,
                                        stop=True,
                                        tile_position=(pb_, 0),
                                    )
                            exs = []
                            for h01 in range(2):
                                ex = pde.tile(
                                    [P, KG, 512],
                                    mybir.dt.bfloat16,
                                    tag=f"ex{h01}",
                                    name=f"ex{h01}",
                                )
                                nc.scalar.activation(ex, sss[h01], AF.Exp)
                                exs.append(ex)
                            for h01 in range(2):
                                for j in range(KG):
                                    kt = ktg * KG + j
                                    nc.tensor.matmul(
                                        pv[h01],
                                        Vt[kt][:, 2 * hp + h01, :],
                                        exs[h01][:, j, :],
                                        start=(kt == 0),
                                        stop=(kt == NKT - 1),
                                    )
                        for h01 in range(2):
                            pb_ = h01 * DK
                            rc = pdr.tile([1, 512], MM_DT, tag="rc", name="rc")
                            with nc.allow_low_precision(
                                reason="1/denom feeds f32r broadcast matmul"
                            ):
                                nc.vector.reciprocal(rc, pv[h01][DK : DK + 1, :])
                            rbp = psR.tile([DK, 512], F32, tag="rbp", name="rbp")
                            nc.tensor.matmul(rbp, ones_t, rc, start=True, stop=True)
                            dst = xo_t[pb_ : pb_ + DK, qt * 512 : (qt + 1) * 512]
                            nc.vector.tensor_copy(dst, pv[h01][0:DK, :])
                            nc.vector.tensor_mul(dst, dst, rbp)
                            nc.vector.tensor_scalar_add(
                                dst, dst, bv_p[pb_ : pb_ + DK, hp : hp + 1]
                            )

                kproj(0)
                for hp in range(NHP):
                    if hp + 1 < NHP:
                        kproj(hp + 1)
                    attn(hp)

            pdx_cm.__exit__(None, None, None)

        # ---- Phase E: out = LN(x_o @ Wo^T + bo + q)  (bo pre-added to qres)
        with (
            tc.tile_pool(name="pe1", bufs=NDT) as pe1,
            tc.tile_pool(name="pec", bufs=1) as pec,
            tc.tile_pool(name="peq", bufs=2) as peq,
            tc.tile_pool(name="pey", bufs=4) as pey,
            tc.tile_pool(name="pst", bufs=8) as pst,
            tc.tile_pool(name="psE", bufs=4, space="PSUM") as psE,
        ):
            g_b = pec.tile([P, D], F32)
            b_b = pec.tile([P, D], F32)
            eps_t = pec.tile([P, 1], F32)
            nc.sync.dma_start(g_b, gv[:].partition_broadcast(P))
            nc.sync.dma_start(b_b, bv2[:].partition_broadcast(P))
            nc.vector.memset(eps_t, 1e-5)
            xo = XO
            for rt in range(NRT_O):
                qr = peq.tile([P, D], F32)
                nc.sync.dma_start(qr, qres[rt * P : (rt + 1) * P, :])
                y = pey.tile([P, D], F32)
                for o2 in range(2):
                    ps = psE.tile([P, 512], F32)
                    for hp in range(NOT):
                        nc.tensor.matmul(
                            ps,
                            xo[hp][:, rt * P : (rt + 1) * P],
                            wo[hp][:, o2 * 512 : (o2 + 1) * 512],
                            start=(hp == 0),
                            stop=(hp == NOT - 1),
                        )
                    nc.vector.tensor_add(
                        y[:, o2 * 512 : (o2 + 1) * 512],
                        ps,
                        qr[:, o2 * 512 : (o2 + 1) * 512],
                    )
                stats = pst.tile([P, 2, 6], F32)
                for sg in range(2):
                    nc.vector.bn_stats(
                        stats[:, sg, :], y[:, sg * 512 : (sg + 1) * 512]
                    )
                mv = pst.tile([P, 2], F32)
                nc.vector.bn_aggr(mv, stats)
                std = pst.tile([P, 1], F32)
                nc.scalar.activation(std, mv[:, 1:2], AF.Sqrt, bias=eps_t)
                rstd = pst.tile([P, 1], F32)
                nc.vector.reciprocal(rstd, std)
                nc.vector.tensor_scalar(
                    y,
                    y,
                    mv[:, 0:1],
                    rstd,
                    op0=ALU.subtract,
                    op1=ALU.mult,
                )
                eng = nc.vector if rt % 2 == 0 else nc.gpsimd
                eng.tensor_mul(y, y, g_b)
                eng.tensor_add(y, y, b_b)
                nc.sync.dma_start(out[rt * P : (rt + 1) * P, :], y)
        pwo_cm.__exit__(None, None, None)
        pxo_cm.__exit__(None, None, None)
        loop_cm.__exit__(None, None, None)
    _split_sync_waits(nc)
    return nc


_NC = None


def _get_nc():
    global _NC
    if _NC is None:
        _NC = build_nc()
    return _NC


def prepare_in_maps(q, k, v, Wq, bq, Wk, bk, Wv, bv, Wo, bo, ln_g, ln_b):
    f = np.float32
    q = np.asarray(q, f)
    k = np.asarray(k, f)
    v = np.asarray(v, f)
    scale = 1.0 / np.sqrt(np.float32(DK))
    WqT = np.ascontiguousarray((np.asarray(Wq, f).T * scale).astype(ml_dtypes.bfloat16))
    WkT = np.ascontiguousarray(np.asarray(Wk, f).T.astype(ml_dtypes.bfloat16))
    WvT = np.ascontiguousarray(np.asarray(Wv, f).T.astype(ml_dtypes.bfloat16))
    WoT = np.ascontiguousarray(np.asarray(Wo, f).T.astype(ml_dtypes.bfloat16))
    bq_s = np.asarray(bq, f) * scale
    common = {
        "WqT": WqT,
        "WkT": WkT,
        "WvT": WvT,
        "WoT": WoT,
        "bq": bq_s,
        "bk": np.asarray(bk, f),
        "bv": np.asarray(bv, f),
        "ln_g": np.asarray(ln_g, f),
        "ln_b": np.asarray(ln_b, f),
        "onesv": np.ones((P, NRT_V * H), ml_dtypes.bfloat16),
        "onesf": np.ones(DK, np.float32),
    }
    in_maps = []
    for c in range(8):
        b_, half = divmod(c, 2)
        qs = q[b_, half * M : (half + 1) * M, :]
        qres_c = qs + np.asarray(bo, f)[None, :]
        in_maps.append(
            dict(
                common,
                xqT=np.ascontiguousarray(qs.T.astype(ml_dtypes.bfloat16)),
                xkT=np.ascontiguousarray(k[b_].T.astype(ml_dtypes.bfloat16)),
                xvT=np.ascontiguousarray(v[b_].T.astype(ml_dtypes.bfloat16)),
                qres=np.ascontiguousarray(qres_c),
            )
        )
    return in_maps


def kernel(q, k, v, Wq, bq, Wk, bk, Wv, bv, Wo, bo, ln_g, ln_b):
    nc = _get_nc()
    in_maps = prepare_in_maps(q, k, v, Wq, bq, Wk, bk, Wv, bv, Wo, bo, ln_g, ln_b)
    res = run_bass_kernel_spmd(nc, in_maps, core_ids=list(range(8)))
    out = np.empty((B, S, D), np.float32)
    for c in range(8):
        b_, half = divmod(c, 2)
        out[b_, half * M : (half + 1) * M, :] = res.results[c]["out"]
    return out
